# revision 25
# baseline (speedup 1.0000x reference)
"""CosFormer causal attention — Trainium2 Bass kernel, 8 NeuronCores.

Sharding: core i = (batch b = i//4, head-group g = i%4 covering heads 2g, 2g+1).
Each core computes the qkv projection for its two heads, chunked causal linear
attention (cos/sin feature channels), and a partial output projection over its
128 context channels. The host unshards by summing the 4 per-core partials of
each batch (the output projection's contraction is sharded over heads) and
adding b_out.

Key layout/perf choices:
- Per-head q/k features in [feat, t] layout as one [128, T] float32r tile
  (rows 0:64 = relu(.)*cos_t, 64:128 = relu(.)*sin_t), produced by projecting
  with duplicated weight columns (PE cost scales with the moving dim, not M).
- float32r everywhere on the PE: 1 cycle/row when the moving dim is >= 256
  (vs 4 for plain fp32), fp32 PSUM accumulation.
- Attention runs in 256-wide query super-chunks: two 128-wide key stripes are
  scored against the full 256 query band (moving dim 256) and masked, so every
  attention matmul streams at full rate. A [2d, d+2] state carries the prefix
  between super-chunks.
- Normalization: norm row -> PE-transpose to a [t, 1] column -> 1-elem/lane
  reciprocal -> applied per-partition to the per-head out-projection partials.

Fully self-contained: hardcodes B=2, T=1024, E=512, H=8.
"""

import math
from contextlib import ExitStack

import numpy as np

import concourse.bass as bass
import concourse.mybir as mybir
import concourse.tile as tile
from concourse.bass_utils import run_bass_kernel_spmd
from concourse.vector_clock import ScopedClock

B, T, E = 2, 1024, 512
H, D = 8, 64
S = 128            # key stripe size
SC = 256           # query super-chunk size
NSC = T // SC      # 4
F32 = mybir.dt.float32
F32R = mybir.dt.float32r
EPS = 1e-6


def _install_drain_patch():
    """This walrus build rejects a Drain carrying >1 sem wait. Split the
    Tile-exit drain's waits across single-wait SP nops."""
    if getattr(tile.TileContext, "_drain_patch_installed", False):
        return

    def _patched(self, tick_clock, wait_clock):
        nc = self.nc
        pre = nc.sync.nop(nofuse=True)
        wait_clock.add_sem_waits(pre.ins, ScopedClock({None: tick_clock.global_clock}))
        waits = list(pre.ins.sync_info.on_wait or []) if pre.ins.sync_info else []
        if len(waits) > 1:
            pre.ins.sync_info.on_wait = waits[:1]
            for w in waits[1:]:
                n = nc.sync.nop(nofuse=True)
                if n.ins.sync_info is None:
                    n.ins.sync_info = mybir.SyncInfo(on_wait=[w], on_update=[])
                else:
                    n.ins.sync_info.on_wait = [w]
        nc.sync.drain()
        nc.all_engine_barrier()
        popped = nc._tile_sem_poison_stack.pop()
        assert popped is self._sem_poison
        nc.clear_and_free_semaphores(list(self.sems.allocated().values()))
        nc.all_engine_barrier()

    tile.TileContext._drain_and_barrier = _patched
    tile.TileContext._drain_patch_installed = True


def _split_multi_waits(nc):
    """This walrus build only codegens ONE sync-wait command per instruction.
    Move excess waits onto same-engine NoOps inserted just before."""
    ctr = [0]

    def _mk_nop(engine, wait):
        ctr[0] += 1
        return mybir.InstNoOp(
            name=f"I-waitnop{ctr[0]}",
            engine=engine,
            ins=[],
            outs=[],
            sync_info=mybir.SyncInfo(on_wait=[wait], on_update=[]),
        )

    for f in nc.m.functions:
        for bb in f.blocks:
            new_insts = []
            for inst in bb.instructions:
                si = inst.sync_info
                waits = list(si.on_wait) if si and si.on_wait else []
                if len(waits) > 1:
                    for w in waits[:-1]:
                        new_insts.append(_mk_nop(inst.engine, w))
                    si.on_wait = waits[-1:]
                new_insts.append(inst)
            bb.instructions[:] = new_insts


def build_program() -> bass.Bass:
    _install_drain_patch()
    nc = bass.Bass()

    # wqkf: duplicated weight cols [qf_h0 | qf_h1 | kf_h0 | kf_h1], each 128 wide
    xt = nc.declare_dram_parameter("xt", [E, T], F32R, isOutput=False)        # x[b].T
    wqkf = nc.declare_dram_parameter("wqkf", [E, 512], F32R, isOutput=False)
    wvt = nc.declare_dram_parameter("wvt", [E, 128], F32R, isOutput=False)    # [v0 v1].T
    bqkf = nc.declare_dram_parameter("bqkf", [640], F32, isOutput=False)      # dup'd qk biases + v bias
    csrep = nc.declare_dram_parameter("csrep", [128, T], F32, isOutput=False)  # [cos;sin]
    w2 = nc.declare_dram_parameter("w2", [128, E], F32R, isOutput=False)
    identin = nc.declare_dram_parameter("identin", [128, 128], F32R, isOutput=False)
    m0in = nc.declare_dram_parameter("m0in", [S, SC], F32, isOutput=False)    # [tri | ones]
    m1in = nc.declare_dram_parameter("m1in", [S, SC], F32, isOutput=False)    # [zeros | tri]
    out = nc.declare_dram_parameter("out", [T, E], F32, isOutput=True)

    with tile.TileContext(nc) as tc, ExitStack() as ctx:
        singles = ctx.enter_context(tc.tile_pool(name="singles", bufs=1))
        kf_pool = ctx.enter_context(tc.tile_pool(name="kf", bufs=4))
        atm_pool = ctx.enter_context(tc.tile_pool(name="atm", bufs=3))
        osb_pool = ctx.enter_context(tc.tile_pool(name="osb", bufs=2))
        nrm_pool = ctx.enter_context(tc.tile_pool(name="nrm", bufs=4))
        pp_big = ctx.enter_context(tc.tile_pool(name="pp_big", bufs=2, space="PSUM"))
        pp_mm = ctx.enter_context(tc.tile_pool(name="pp_mm", bufs=2, space="PSUM"))
        pp_psn = ctx.enter_context(tc.tile_pool(name="pp_psn", bufs=1, space="PSUM"))
        pp_kt = ctx.enter_context(tc.tile_pool(name="pp_kt", bufs=1, space="PSUM"))
        pp_cs = ctx.enter_context(tc.tile_pool(name="pp_cs", bufs=2, space="PSUM"))

        # ---- constant / input tiles (big ones split per k-block) ---------
        xt_s = singles.tile([128, 4, T], F32R)
        xt_r = xt.rearrange("(kk p) t -> p kk t", p=128)
        for kk in range(4):
            nc.sync.dma_start(out=xt_s[:, kk, :], in_=xt_r[:, kk, :])
        wqkf_s = singles.tile([128, 4, 512], F32R)
        wqkf_r = wqkf.rearrange("(kk p) c -> p kk c", p=128)
        for kk in range(4):
            nc.sync.dma_start(out=wqkf_s[:, kk, :], in_=wqkf_r[:, kk, :])
        wvt_s = singles.tile([128, 4, 128], F32R)
        nc.sync.dma_start(out=wvt_s, in_=wvt.rearrange("(kk p) c -> p kk c", p=128))
        w2h = []
        for h in range(2):
            t_ = singles.tile([D, E], F32R, name=f"w2h{h}")
            nc.sync.dma_start(out=t_, in_=w2[h * D:(h + 1) * D, :])
            w2h.append(t_)
        cs_s = singles.tile([128, T], F32)
        nc.sync.dma_start(out=cs_s, in_=csrep[:, :])
        biases = []
        for bi in range(4):
            t_ = singles.tile([128, 1], F32, name=f"bias{bi}")
            nc.sync.dma_start(out=t_, in_=bqkf[bi * 128:(bi + 1) * 128, None])
            biases.append(t_)
        bias_v = singles.tile([128, 1], F32, name="bias_v")
        nc.sync.dma_start(out=bias_v, in_=bqkf[512:640, None])

        ident = singles.tile([128, 128], F32R)
        nc.sync.dma_start(out=ident, in_=identin[:, :])
        m0_s = singles.tile([S, SC], F32)
        nc.sync.dma_start(out=m0_s, in_=m0in[:, :])
        m1_s = singles.tile([S, SC], F32)
        nc.sync.dma_start(out=m1_s, in_=m1in[:, :])
        eps_t = singles.tile([1, 1], F32, name="eps_t")
        nc.vector.memset(eps_t, EPS)
        onesz_col = singles.tile([128, 2], F32, name="onesz_col")
        nc.vector.memset(onesz_col[:, 0:1], 1.0)
        nc.vector.memset(onesz_col[:, 1:2], 0.0)

        # per-head stacked feature tiles [cos;sin] x t
        qfT = [singles.tile([128, T], F32R, name=f"qfT{h}") for h in range(2)]
        kfT = [singles.tile([128, T], F32R, name=f"kfT{h}") for h in range(2)]
        vT = singles.tile([128, T], F32R, name="vT")
        state = [singles.tile([128, D + 2], F32R, name=f"state{h}") for h in range(2)]
        # persistent V' ring: [head][stripe], ones/pad cols written once
        vp_ring = [[singles.tile([S, D + 2], F32R, name=f"vpr{h}_{ci}")
                    for ci in range(2)] for h in range(2)]
        for h in range(2):
            for ci in range(2):
                nc.scalar.copy(vp_ring[h][ci][:, D:D + 2], onesz_col)

        # ---- q/k features in [feat, t] layout ----------------------------
        # block bi: 0=qf_h0, 1=qf_h1, 2=kf_h0, 3=kf_h1
        for bi, dst in ((0, qfT[0]), (1, qfT[1]), (2, kfT[0]), (3, kfT[1])):
            for th in range(2):
                tslh = slice(th * 512, (th + 1) * 512)
                ps = pp_big.tile([128, 512], F32, tag="big", name=f"psB{bi}_{th}")
                for kk in range(4):
                    nc.tensor.matmul(
                        ps,
                        wqkf_s[:, kk, bi * 128:(bi + 1) * 128],
                        xt_s[:, kk, tslh],
                        start=(kk == 0),
                        stop=(kk == 3),
                    )
                nc.scalar.activation(
                    out=dst[:, tslh],
                    in_=ps,
                    func=mybir.ActivationFunctionType.Relu,
                    bias=biases[bi],
                    scale=1.0,
                )
                nc.vector.tensor_mul(dst[:, tslh], dst[:, tslh], cs_s[:, tslh])

        # ---- v^T projection, [col, t] layout (bias fused in ACT) ---------
        for th in range(2):
            tslh = slice(th * 512, (th + 1) * 512)
            ps = pp_big.tile([128, 512], F32, tag="big", name=f"psV{th}")
            for kk in range(4):
                nc.tensor.matmul(
                    ps,
                    wvt_s[:, kk, :],
                    xt_s[:, kk, tslh],
                    start=(kk == 0),
                    stop=(kk == 3),
                )
            nc.scalar.activation(
                out=vT[:, tslh],
                in_=ps,
                func=mybir.ActivationFunctionType.Identity,
                bias=bias_v,
                scale=1.0,
            )

        # ---- attention, 256-wide query super-chunks ----------------------
        for sc in range(NSC):
            t0 = sc * SC
            band = slice(t0, t0 + SC)
            sub = [slice(t0, t0 + S), slice(t0 + S, t0 + 2 * S)]

            # stripe transposes: kfT/vT [*, t] -> [t, *] per 128-stripe
            kfeat = [[None, None], [None, None]]  # [ci][h]
            vp = [[None, None], [None, None]]     # [ci][h]
            for ci in range(2):
                ps_kt = pp_kt.tile([128, 384], F32R, tag="kt", name=f"pskt{sc}_{ci}")
                for h in range(2):
                    kfeat[ci][h] = kf_pool.tile(
                        [S, 128], F32R, tag=f"kf{h}", name=f"kfeat{sc}_{ci}_{h}")
                    nc.tensor.transpose(
                        ps_kt[:, h * 128:(h + 1) * 128], kfT[h][:, sub[ci]], ident)
                nc.vector.tensor_copy(kfeat[ci][0], ps_kt[:, 0:128])
                nc.scalar.copy(kfeat[ci][1], ps_kt[:, 128:256])
                nc.tensor.transpose(ps_kt[:, 256:384], vT[:, sub[ci]], ident)
                for h in range(2):
                    vp[ci][h] = vp_ring[h][ci]
                nc.vector.tensor_copy(vp[ci][0][:, 0:D], ps_kt[:, 256:256 + D])
                nc.scalar.copy(vp[ci][1][:, 0:D], ps_kt[:, 256 + D:256 + 2 * D])

            ps_o = [[None, None], [None, None]]   # [ci][h]
            ncol = [[None, None], [None, None]]   # [ci][h]
            for h in range(2):
                # key-stripe scores against the whole query band, masked
                atm = []
                for ci, m_s in ((0, m0_s), (1, m1_s)):
                    ps_a = pp_mm.tile([S, SC], F32, tag="mm", name=f"psa{sc}_{ci}_{h}")
                    nc.tensor.matmul(ps_a, kfT[h][:, sub[ci]], qfT[h][:, band],
                                     start=True, stop=True)
                    a_t = atm_pool.tile([S, SC], F32R, tag="atm", name=f"atm{sc}_{ci}_{h}")
                    nc.vector.tensor_mul(a_t, ps_a, m_s)
                    atm.append(a_t)

                # ctx^T (+norm row 64) = prefix-state inter + two stripe intras
                ps_c = pp_cs.tile([D + 2, SC], F32, tag="cs", name=f"psc{sc}_{h}")
                if sc > 0:
                    nc.tensor.matmul(ps_c, state[h], qfT[h][:, band], start=True, stop=False)
                    nc.tensor.matmul(ps_c, vp[0][h], atm[0], start=False, stop=False)
                    nc.tensor.matmul(ps_c, vp[1][h], atm[1], start=False, stop=True)
                else:
                    nc.tensor.matmul(ps_c, vp[0][h], atm[0], start=True, stop=False)
                    nc.tensor.matmul(ps_c, vp[1][h], atm[1], start=False, stop=True)

                # state += Kf^T V' over both stripes
                ps_s = pp_cs.tile([128, D + 2], F32, tag="cs", name=f"pss{sc}_{h}")
                nc.tensor.matmul(ps_s, kfeat[0][h], vp[0][h], start=True, stop=False)
                nc.tensor.matmul(ps_s, kfeat[1][h], vp[1][h], start=False, stop=True)
                if sc == 0:
                    nc.vector.tensor_copy(state[h], ps_s)
                else:
                    nc.vector.tensor_add(state[h], state[h], ps_s)

                # norm row -> [t,1] columns (PE transpose) -> reciprocal
                nrow = nrm_pool.tile([1, SC], F32R, tag="nrow", name=f"nrow{sc}_{h}")
                nc.scalar.activation(out=nrow, in_=ps_c[D:D + 1, :],
                                     func=mybir.ActivationFunctionType.Identity,
                                     bias=eps_t[0:1, 0:1], scale=1.0)
                for ci in range(2):
                    ps_n = pp_psn.tile([S, 2], F32R, tag="psn", name=f"psn{sc}_{ci}_{h}")
                    nc.tensor.transpose(ps_n, nrow[:, ci * S:(ci + 1) * S], ident[0:1, 0:2])
                    nc_t = nrm_pool.tile([S, 1], F32, tag="ncol", name=f"ncol{sc}_{ci}_{h}")
                    nc.vector.reciprocal(nc_t, ps_n[:, 0:1])
                    ncol[ci][h] = nc_t

                # unnormalized ctx -> SBUF; per-stripe per-head out-projection
                ctxu = nrm_pool.tile([D, SC], F32R, tag="ctxu", name=f"ctxu{sc}_{h}")
                nc.scalar.copy(ctxu, ps_c[0:D, :])
                for ci in range(2):
                    ps = pp_big.tile([128, E], F32, tag="big", name=f"pso{sc}_{ci}_{h}")
                    nc.tensor.matmul(ps, ctxu[:, ci * S:(ci + 1) * S], w2h[h],
                                     start=True, stop=True)
                    ps_o[ci][h] = ps

            # scale by 1/norm (per-partition) and combine heads
            for ci in range(2):
                o_s = osb_pool.tile([128, E], F32, tag="osb", name=f"os{sc}_{ci}")
                nc.scalar.activation(out=o_s, in_=ps_o[ci][0],
                                     func=mybir.ActivationFunctionType.Copy,
                                     scale=ncol[ci][0])
                nc.vector.scalar_tensor_tensor(
                    out=o_s, in0=ps_o[ci][1], scalar=ncol[ci][1], in1=o_s,
                    op0=mybir.AluOpType.mult, op1=mybir.AluOpType.add,
                )
                nc.sync.dma_start(out=out[sub[ci], :], in_=o_s)

    _split_multi_waits(nc)
    return nc


_PROGRAM = None


def _get_program():
    global _PROGRAM
    if _PROGRAM is None:
        _PROGRAM = build_program()
    return _PROGRAM


def _make_in_maps(x, w_qkv, b_qkv, w_out):
    pos = np.arange(T, dtype=np.float32)
    ang = (math.pi / 2) * pos / T
    cosw = np.cos(ang).astype(np.float32)
    sinw = np.sin(ang).astype(np.float32)
    csrep = np.concatenate([
        np.broadcast_to(cosw[None, :], (D, T)),
        np.broadcast_to(sinw[None, :], (D, T)),
    ], 0).astype(np.float32)
    tri = np.triu(np.ones((S, S), np.float32))
    m0 = np.concatenate([tri, np.ones((S, S), np.float32)], 1)
    m1 = np.concatenate([np.zeros((S, S), np.float32), tri], 1)

    in_maps = []
    for i in range(8):
        b, g = divmod(i, 4)
        h0, h1 = 2 * g, 2 * g + 1
        wq = lambda h: w_qkv[h * D:(h + 1) * D]
        wk = lambda h: w_qkv[E + h * D:E + (h + 1) * D]
        wv = lambda h: w_qkv[2 * E + h * D:2 * E + (h + 1) * D]
        bq = lambda h: b_qkv[h * D:(h + 1) * D]
        bk = lambda h: b_qkv[E + h * D:E + (h + 1) * D]
        bv = lambda h: b_qkv[2 * E + h * D:2 * E + (h + 1) * D]
        hcols = np.r_[h0 * D:(h0 + 1) * D, h1 * D:(h1 + 1) * D]
        wqkf = np.concatenate([
            wq(h0), wq(h0), wq(h1), wq(h1), wk(h0), wk(h0), wk(h1), wk(h1)
        ], 0).T
        bqkf = np.concatenate([
            bq(h0), bq(h0), bq(h1), bq(h1), bk(h0), bk(h0), bk(h1), bk(h1),
            bv(h0), bv(h1)
        ])
        in_maps.append({
            "xt": np.ascontiguousarray(x[b].T),
            "wqkf": np.ascontiguousarray(wqkf),
            "wvt": np.ascontiguousarray(np.concatenate([wv(h0), wv(h1)], 0).T),
            "bqkf": np.ascontiguousarray(bqkf),
            "csrep": csrep,
            "w2": np.ascontiguousarray(w_out[:, hcols].T),
            "identin": np.eye(128, dtype=np.float32),
            "m0in": m0,
            "m1in": m1,
        })
    return in_maps


def run(inputs, trace=False):
    x = np.asarray(inputs["x"], dtype=np.float32)
    w_qkv = np.asarray(inputs["w_qkv"], dtype=np.float32)
    b_qkv = np.asarray(inputs["b_qkv"], dtype=np.float32)
    w_out = np.asarray(inputs["w_out"], dtype=np.float32)
    b_out = np.asarray(inputs["b_out"], dtype=np.float32)

    nc = _get_program()
    in_maps = _make_in_maps(x, w_qkv, b_qkv, w_out)
    res = run_bass_kernel_spmd(nc, in_maps, list(range(8)), trace=trace)

    out = np.empty((B, T, E), dtype=np.float32)
    for b in range(B):
        acc = res.results[4 * b]["out"].astype(np.float32)
        for g in range(1, 4):
            acc = acc + res.results[4 * b + g]["out"]
        out[b] = acc + b_out[None, :]
    return out, res


def kernel(**inputs) -> np.ndarray:
    out, _ = run(inputs, trace=False)
    return out
